# revision 28
# baseline (speedup 1.0000x reference)
"""Trainium2 Bass kernel: causal self-attention with RoPE.

Sharding: tensor-parallel on the head axis. 16 heads over 8 cores = 2 heads
per core. Each core computes q/k/v projections for its 2 heads (from the
full, replicated input), runs causal attention for those heads over both
batch elements, and applies its slice of the output projection, producing a
partial [B*S, E] output. The host sums the 8 partials (the "all-reduce").

Device-side layout choices:
  - x is passed pre-transposed ([E, B*S], bf16) so projections need no
    on-device transpose.
  - q and k are produced "d-major" (qT [2*64, B*S]); scores are computed
    transposed (S_T[k, q] = k_tile @ qT) so that P@V needs no transposes:
    O_T = [v | 1].T @ P_T, which also yields the softmax denominator as
    row 64 of the PSUM accumulator. Softmax uses no max-subtraction (max
    logit ~11 for this problem, exp is safe in fp32).
  - Both heads' score blocks live in one 2-bank PSUM tile so the exp runs
    as a single fused op over [128, 2, nj].
  - RoPE: q' = q * cos + shuffle(q) * sin_signed, where shuffle swaps
    adjacent partitions (stream_shuffle) and the sign lives in the host
    precomputed sin table.
  - Causal masking: after exp, the diagonal-crossing 128-wide region is
    multiplied by a 0/1 bf16 triangular mask (SBUF-only DVE op, 2x mode);
    fully-masked columns are never computed.
  - Emission is phase-pipelined per batch: batch-1 projections and
    batch-0 output-projection tiles are interleaved into the attention
    loops as PE filler work so the tensor engine stays busy (HAM-warm)
    while the scalar engine works through the exp stream.
"""

import functools

import numpy as np
import ml_dtypes

import concourse.bass as bass
import concourse.mybir as mybir
import concourse.tile as tile
from concourse import bacc
from concourse.bass_utils import run_bass_kernel_spmd

F32 = mybir.dt.float32
BF16 = mybir.dt.bfloat16
BF = ml_dtypes.bfloat16

E = 1024
HD = 64
N_CORES = 8
ROPE_BASE = 10000.0


def _split(lst, n):
    """Split lst into n contiguous groups (some may be empty)."""
    out = []
    base = 0
    for i in range(n):
        take = (len(lst) - base + (n - i - 1)) // (n - i)
        out.append(lst[base:base + take])
        base += take
    return out


def _build(seq: int, nb: int) -> bacc.Bacc:
    TS = nb * seq                 # total sequence columns (batches concatenated)
    QC = min(512, seq)            # q-chunk width for attention
    NQC = seq // QC               # q-chunks per batch
    NKTB = seq // 128             # k-tiles per batch
    NET = E // 128                # contraction tiles = 8
    PCB = min(512, seq)           # per-batch projection s-chunk
    NPCB = seq // PCB
    NSTB = seq // 128             # s-tiles per batch

    nc = bacc.Bacc(
        "TRN2",
        target_bir_lowering=False,
        debug=False,
        enable_asserts=False,
        num_devices=N_CORES,
    )

    xT_d = nc.dram_tensor("xT", [E, TS], BF16, kind="ExternalInput").ap()
    wq_d = nc.dram_tensor("wqT", [E, 128], BF16, kind="ExternalInput").ap()
    wk_d = nc.dram_tensor("wkT", [E, 128], BF16, kind="ExternalInput").ap()
    wv_d = nc.dram_tensor("wvT", [E, 128], BF16, kind="ExternalInput").ap()
    wo_d = nc.dram_tensor("woT", [128, E], BF16, kind="ExternalInput").ap()
    cos_d = nc.dram_tensor("cosT", [128, seq], BF16, kind="ExternalInput").ap()
    sin_d = nc.dram_tensor("sinT", [128, seq], BF16, kind="ExternalInput").ap()
    tri_d = nc.dram_tensor("tri", [128, 128], BF16, kind="ExternalInput").ap()
    out_d = nc.dram_tensor("out_p", [TS, E], F32, kind="ExternalOutput").ap()
    _nqc = seq // min(512, seq)
    _qc = min(512, seq)
    recd = [nc.dram_tensor(f"rec_scratch{b}", [2 * _nqc, _qc], F32).ap()
            for b in range(nb)]

    with tile.TileContext(nc) as tc:
        with (
            tc.tile_pool(name="persist", bufs=1) as persist,
            tc.tile_pool(name="pt", bufs=NKTB) as ptp,
            tc.tile_pool(name="ob", bufs=4) as obp,
            tc.tile_pool(name="rec", bufs=4) as recp,
            tc.tile_pool(name="ps_big", bufs=2, space="PSUM") as psb,
            tc.tile_pool(name="ps_ov", bufs=3, space="PSUM") as psov,
            tc.tile_pool(name="ps_e", bufs=1, space="PSUM") as pse,
        ):
            def T(shape, dtype, name):
                return persist.tile(shape, dtype, name=name, tag=name)

            # ---- constants / weights
            wq_s = T([128, NET, 128], BF16, "wq_s")
            wk_s = T([128, NET, 128], BF16, "wk_s")
            wv_s = T([128, NET, 128], BF16, "wv_s")
            wo_s = T([128, E], BF16, "wo_s")
            cos_s = T([128, seq], BF16, "cos_s")
            sin_s = T([128, seq], BF16, "sin_s")
            tri_s = T([128, 128], BF16, "tri_s")
            nc.sync.dma_start(out=wq_s, in_=wq_d.rearrange("(t p) d -> p t d", p=128))
            nc.sync.dma_start(out=wk_s, in_=wk_d.rearrange("(t p) d -> p t d", p=128))
            nc.sync.dma_start(out=wv_s, in_=wv_d.rearrange("(t p) d -> p t d", p=128))
            nc.sync.dma_start(out=wo_s, in_=wo_d)
            nc.sync.dma_start(out=cos_s, in_=cos_d)
            nc.sync.dma_start(out=sin_s, in_=sin_d)
            nc.sync.dma_start(out=tri_s, in_=tri_d)

            # ---- PE warm-up while input DMAs stream (HAM ramps at ~3.4us
            # of sustained activity; these dummy matmuls buy 2.4GHz for the
            # projection phase)
            wu = psb.tile([128, 512], F32, tag="psb", name="warmup")

            def warm(n):
                for _ in range(n):
                    nc.tensor.matmul(
                        wu, lhsT=wq_s[:, 0, :],
                        rhs=wq_s[:, :, :].rearrange("p t d -> p (t d)")[:, 0:512],
                        start=True, stop=True)

            warm(24)

            # ---- resident input, split into per-chunk column tiles so the
            # first projection is gated on ~1MB, not the whole input
            xts4 = {}
            for b in range(nb):
                for pc in range(NPCB):
                    for et in range(NET):
                        xt = T([128, PCB], BF16, f"xt{et}_{b}_{pc}")
                        nc.sync.dma_start(
                            out=xt,
                            in_=xT_d[et * 128:(et + 1) * 128,
                                     b * seq + pc * PCB:
                                     b * seq + (pc + 1) * PCB])
                        xts4[(et, b, pc)] = xt

            qT = T([128, TS], BF16, "qT")
            kT = T([128, TS], BF16, "kT")
            vo = T([128, nb * NKTB, 130], BF16, "vo")   # [vA|1|vB|1] per k-tile
            oT = T([128, TS], BF16, "oT")
            nc.gpsimd.memset(vo, 1.0)

            # ---------- emission helpers ----------
            def proj_qk_pieces(b, pc):
                """Micro-tasks (~2 MMs each) for one q/k projection chunk."""
                cols = slice(b * seq + pc * PCB, b * seq + (pc + 1) * PCB)
                xcols = slice(pc * PCB, (pc + 1) * PCB)
                pieces = []
                state = {}
                for wi, (w_s, dst) in enumerate(((wq_s, qT), (wk_s, kT))):
                    for e0 in range(0, NET, 2):
                        def piece(wi=wi, w_s=w_s, dst=dst, e0=e0):
                            if e0 == 0:
                                state[wi] = pse.tile(
                                    [128, PCB], F32, tag="pse",
                                    name=f"qk{b}_{pc}_{wi}")
                            ps = state[wi]
                            for et in (e0, e0 + 1):
                                nc.tensor.matmul(
                                    ps, lhsT=w_s[:, et, :],
                                    rhs=xts4[(et, b, pc)],
                                    start=(et == 0), stop=(et == NET - 1),
                                )
                            if e0 + 2 == NET:
                                if b == 0:
                                    nc.scalar.copy(out=dst[:, cols], in_=ps)
                                else:
                                    nc.vector.tensor_copy(out=dst[:, cols],
                                                          in_=ps)
                                if wi == 1:
                                    rope(b, pc)
                        pieces.append(piece)
                return pieces

            def proj_qk(b, pc):
                for p in proj_qk_pieces(b, pc):
                    p()

            def proj_v_pieces(b, st):
                gst = b * NSTB + st
                state = {}
                pieces = []
                for e0 in range(0, NET, 4):
                    def piece(e0=e0):
                        if e0 == 0:
                            state[0] = pse.tile([128, 128], F32, tag="pse",
                                                name=f"v{gst}")
                        ps = state[0]
                        per = PCB // 128
                        vpc = st // per
                        vc0 = (st % per) * 128
                        for et in range(e0, e0 + 4):
                            nc.tensor.matmul(
                                ps,
                                lhsT=xts4[(et, b, vpc)][:, vc0:vc0 + 128],
                                rhs=wv_s[:, et, :],
                                start=(et == 0), stop=(et == NET - 1),
                            )
                        if e0 + 4 == NET:
                            base = vo[:, gst, :]
                            dst = bass.AP(
                                tensor=base.tensor, offset=base.offset,
                                ap=[list(base.ap[0]), [65, 2], [1, 64]])
                            nc.vector.tensor_copy(
                                out=dst,
                                in_=ps.rearrange("p (h d) -> p h d", d=64))
                    pieces.append(piece)
                return pieces

            def proj_v(b, st):
                for p in proj_v_pieces(b, st):
                    p()

            def rope(b, pc=None):
                pcs = range(NPCB) if pc is None else [pc]
                for t, nm in ((qT, "q"), (kT, "k")):
                    for p in pcs:
                        cols = slice(b * seq + p * PCB, b * seq + (p + 1) * PCB)
                        tcols = slice(p * PCB, (p + 1) * PCB)
                        sh = recp.tile([128, PCB], BF16, tag="ropesh",
                                       name=f"sh{nm}{b}_{p}")
                        nc.vector.stream_shuffle(
                            sh, t[:, cols], [i ^ 1 for i in range(32)])
                        nc.vector.tensor_mul(sh, sh, sin_s[:, tcols])
                        nc.vector.tensor_mul(t[:, cols], t[:, cols], cos_s[:, tcols])
                        nc.vector.tensor_add(t[:, cols], t[:, cols], sh)

            pts_cache = {}

            def d1_kj(b, c, kj):
                qbase = c * QC
                gq0 = b * seq + qbase
                o = kj * 128 - qbase
                ro = max(o, 0)
                nj = QC - ro
                kc = b * seq + kj * 128
                ps = psb.tile([128, 2, QC], F32, tag="psb",
                              name=f"ss{b}_{c}_{kj}")
                for h in range(2):
                    rows = slice(h * 64, h * 64 + 64)
                    nc.tensor.matmul(
                        ps[:, h, 0:nj],
                        lhsT=kT[rows, kc:kc + 128],
                        rhs=qT[rows, gq0 + ro:gq0 + QC],
                        start=True, stop=True,
                        tile_position=(h * 64, 0),
                    )
                pt = ptp.tile([128, 2, QC], BF16, tag="pt",
                              name=f"pt{b}_{c}_{kj}")
                nc.scalar.activation(
                    pt[:, :, 0:nj], ps[:, :, 0:nj],
                    mybir.ActivationFunctionType.Exp,
                )
                if o >= 0:
                    tri_b = bass.AP(
                        tensor=tri_s.tensor, offset=tri_s.offset,
                        ap=[list(tri_s.ap[0]), [0, 2], list(tri_s.ap[1])],
                    )
                    nc.gpsimd.tensor_mul(
                        pt[:, :, 0:128], pt[:, :, 0:128], tri_b)
                return pt, ro, nj

            def attn_chunk(b, c, fills):
                qbase = c * QC
                gq0 = b * seq + qbase
                nkt = (qbase + QC) // 128
                ops_ = [psov.tile([65, QC], F32, tag="psov", name=f"o{b}_{c}_{h}")
                        for h in range(2)]
                fq = []
                for kind, idx in fills:
                    if kind == "qk0":
                        fq.extend(proj_qk_pieces(0, idx))
                    elif kind == "v0":
                        fq.extend(proj_v_pieces(0, idx))
                    elif kind == "qk":
                        fq.extend(proj_qk_pieces(1, idx))
                    elif kind == "v":
                        fq.extend(proj_v_pieces(1, idx))
                    else:
                        fq.extend(eproj_pieces(idx))
                for kj in range(nkt):
                    pt, ro, nj = d1_kj(b, c, kj)
                    if kj >= 2:
                        d2_kj(b, c, kj - 2, ops_, nkt)
                    for _ in range(2):
                        if fq:
                            fq.pop(0)()
                    pts_cache[(b, c, kj)] = (pt, ro, nj)
                for kj in range(max(nkt - 2, 0), nkt):
                    d2_kj(b, c, kj, ops_, nkt)
                for p in fq:
                    p()
                return ops_

            def d2_kj(b, c, kj, ops_, nkt):
                pt, ro, nj = pts_cache[(b, c, kj)]
                for h in range(2):
                    nc.tensor.matmul(
                        ops_[h][:, ro:QC],
                        lhsT=vo[:, b * NKTB + kj, h * 65:h * 65 + 65],
                        rhs=pt[:, h, 0:nj],
                        start=(kj == 0), stop=(kj == nkt - 1),
                    )

            def d3(b, c, ops_):
                gq0 = b * seq + c * QC
                dch = recp.tile([64, QC], F32, tag="dch", name=f"dch{b}_{c}")
                nc.gpsimd.memset(dch, 1.0)
                for h in range(2):
                    op = ops_[h]
                    nc.vector.tensor_copy(
                        out=oT[h * 64:h * 64 + 64, gq0:gq0 + QC],
                        in_=op[0:64, 0:QC])
                    nc.vector.tensor_copy(
                        out=dch[32 * h:32 * h + 1, :],
                        in_=op[64:65, 0:QC])
                return dch

            def norm_chunk(b, c, dch):
                gq0 = b * seq + c * QC
                rec = recp.tile([64, QC], F32, tag="rec", name=f"rca{b}_{c}")
                lg = recp.tile([64, QC], F32, tag="lg", name=f"lg{b}_{c}")
                nc.scalar.activation(lg, dch,
                                     mybir.ActivationFunctionType.Ln)
                nc.scalar.activation(rec, lg,
                                     mybir.ActivationFunctionType.Exp,
                                     scale=-1.0)
                for h in range(2):
                    nc.sync.dma_start(
                        out=recd[b][2 * c + h:2 * c + h + 1, :],
                        in_=rec[32 * h:32 * h + 1, :])
                rb = recp.tile([128, QC], F32, tag="rb", name=f"rb{b}_{c}")
                for h in range(2):
                    row = recd[b][2 * c + h:2 * c + h + 1, :]
                    bcast = bass.AP(tensor=row.tensor, offset=row.offset,
                                    ap=[[0, 64], [1, QC]])
                    nc.sync.dma_start(out=rb[h * 64:h * 64 + 64, :], in_=bcast)
                nc.vector.tensor_mul(
                    oT[:, gq0:gq0 + QC], oT[:, gq0:gq0 + QC], rb)

            def eproj_pieces(gst):
                return [lambda ec=ec: eproj_one(gst, ec) for ec in range(E // 512)]

            def eproj(gst):
                for ec in range(E // 512):
                    eproj_one(gst, ec)

            def eproj_one(gst, ec):
                if True:
                    ps = pse.tile([128, 512], F32, tag="pse", name=f"op{gst}_{ec}")
                    nc.tensor.matmul(
                        ps,
                        lhsT=oT[:, gst * 128:(gst + 1) * 128],
                        rhs=wo_s[:, ec * 512:(ec + 1) * 512],
                        start=True, stop=True,
                    )
                    ob = obp.tile([128, 512], F32, tag="ob", name=f"ob{gst}_{ec}")
                    if gst < NSTB:
                        nc.vector.tensor_copy(out=ob, in_=ps)
                    else:
                        nc.scalar.copy(out=ob, in_=ps)
                    nc.sync.dma_start(
                        out=out_d[gst * 128:(gst + 1) * 128, ec * 512:(ec + 1) * 512],
                        in_=ob,
                    )

            # ---------- emission ----------
            for pc in range(NPCB):
                proj_qk(0, pc)
            for st in range(NSTB):
                proj_v(0, st)

            fillers = {c: [] for c in range(NQC)}
            if nb > 1:
                for i, pc in enumerate(range(NPCB)):
                    fillers[min(i, NQC - 1)].append(("qk", pc))
                for i, grp in enumerate(_split(list(range(NSTB)),
                                               max(NQC - 1, 1))):
                    fillers[min(i + 1, NQC - 1)].extend(("v", st) for st in grp)

            for c in range(NQC):
                ops_ = attn_chunk(0, c, fillers.get(c, []))
                dch = d3(0, c, ops_)
                norm_chunk(0, c, dch)

            if nb > 1:
                # batch-1 attention; batch-0 out-proj as PE filler,
                # batch-1 out-proj one chunk behind
                e0_fill = _split(list(range(NSTB)), NQC)
                spc = QC // 128
                cs = list(reversed(range(NQC)))
                for i, c in enumerate(cs):
                    fills = [("e", st) for st in e0_fill[i]]
                    if i >= 1:
                        pc_ = cs[i - 1]
                        fills += [("e", NSTB + st)
                                  for st in range(pc_ * spc, (pc_ + 1) * spc)]
                    ops_ = attn_chunk(1, c, fills)
                    dch = d3(1, c, ops_)
                    norm_chunk(1, c, dch)
                lc = cs[-1]
                for st in range(lc * spc, (lc + 1) * spc):
                    eproj(NSTB + st)
            else:
                for st in range(NSTB):
                    eproj(st)

    nc.compile()
    return nc


@functools.lru_cache(maxsize=2)
def _built(seq: int, nb: int) -> bacc.Bacc:
    return _build(seq, nb)


def _host_tables(seq: int):
    inv = 1.0 / (ROPE_BASE ** (np.arange(0, HD, 2, dtype=np.float32) / HD))
    f = np.outer(np.arange(seq, dtype=np.float32), inv)
    emb = np.concatenate([f, f], axis=-1)        # [S, 64] (concat layout)
    cos = np.cos(emb).T.astype(np.float32)       # [64, S]
    sin = np.sin(emb).T.astype(np.float32)
    sgn = np.where(np.arange(HD) % 2 == 0, -1.0, 1.0).astype(np.float32)
    sin_signed = sin * sgn[:, None]
    cosT = np.concatenate([cos, cos], axis=0).astype(BF)       # [128, S]
    sinT = np.concatenate([sin_signed, sin_signed], axis=0).astype(BF)
    return cosT, sinT


def make_in_maps(x, Wq, Wk, Wv, Wo):
    x = np.asarray(x, dtype=np.float32)
    B, S, E_ = x.shape
    assert E_ == E
    xT = np.ascontiguousarray(x.reshape(B * S, E_).T).astype(BF)   # [E, B*S]
    cosT, sinT = _host_tables(S)
    i_idx = np.arange(128)
    tri = (i_idx[None, :] >= i_idx[:, None]).astype(BF)  # keep j >= i
    scale = np.float32(HD ** -0.5)
    in_maps = []
    for core in range(N_CORES):
        cols = slice(core * 128, core * 128 + 128)   # heads 2c, 2c+1 dims
        wqT = np.ascontiguousarray((np.asarray(Wq)[cols, :] * scale).T).astype(BF)
        wkT = np.ascontiguousarray(np.asarray(Wk)[cols, :].T).astype(BF)
        wvT = np.ascontiguousarray(np.asarray(Wv)[cols, :].T).astype(BF)
        woT = np.ascontiguousarray(np.asarray(Wo)[:, cols].T).astype(BF)
        in_maps.append(dict(
            xT=xT, wqT=wqT, wkT=wkT, wvT=wvT, woT=woT,
            cosT=cosT, sinT=sinT, tri=tri,
        ))
    return in_maps


def kernel(x, Wq, Wk, Wv, Wo):
    x = np.asarray(x, dtype=np.float32)
    B, S, E_ = x.shape
    nc = _built(S, B)
    in_maps = make_in_maps(x, Wq, Wk, Wv, Wo)
    res = run_bass_kernel_spmd(nc, in_maps, core_ids=list(range(N_CORES)))
    out = np.zeros((B * S, E_), np.float32)
    for r in res.results:
        out += r["out_p"]
    return out.reshape(B, S, E_)


# revision 29
# speedup vs baseline: 1.0356x; 1.0356x over previous
"""Trainium2 Bass kernel: causal self-attention with RoPE.

Sharding: tensor-parallel on the head axis. 16 heads over 8 cores = 2 heads
per core. Each core computes q/k/v projections for its 2 heads (from the
full, replicated input), runs causal attention for those heads over both
batch elements, and applies its slice of the output projection, producing a
partial [B*S, E] output. The host sums the 8 partials (the "all-reduce").

Device-side layout choices:
  - x is passed pre-transposed ([E, B*S], bf16) so projections need no
    on-device transpose.
  - q and k are produced "d-major" (qT [2*64, B*S]); scores are computed
    transposed (S_T[k, q] = k_tile @ qT) so that P@V needs no transposes:
    O_T = [v | 1].T @ P_T, which also yields the softmax denominator as
    row 64 of the PSUM accumulator. Softmax uses no max-subtraction (max
    logit ~11 for this problem, exp is safe in fp32).
  - Both heads' score blocks live in one 2-bank PSUM tile so the exp runs
    as a single fused op over [128, 2, nj].
  - RoPE: q' = q * cos + shuffle(q) * sin_signed, where shuffle swaps
    adjacent partitions (stream_shuffle) and the sign lives in the host
    precomputed sin table.
  - Causal masking: after exp, the diagonal-crossing 128-wide region is
    multiplied by a 0/1 bf16 triangular mask (SBUF-only DVE op, 2x mode);
    fully-masked columns are never computed.
  - Emission is phase-pipelined per batch: batch-1 projections and
    batch-0 output-projection tiles are interleaved into the attention
    loops as PE filler work so the tensor engine stays busy (HAM-warm)
    while the scalar engine works through the exp stream.
"""

import functools

import numpy as np
import ml_dtypes

import concourse.bass as bass
import concourse.mybir as mybir
import concourse.tile as tile
from concourse import bacc
from concourse.bass_utils import run_bass_kernel_spmd

F32 = mybir.dt.float32
BF16 = mybir.dt.bfloat16
BF = ml_dtypes.bfloat16

E = 1024
HD = 64
N_CORES = 8
ROPE_BASE = 10000.0


def _split(lst, n):
    """Split lst into n contiguous groups (some may be empty)."""
    out = []
    base = 0
    for i in range(n):
        take = (len(lst) - base + (n - i - 1)) // (n - i)
        out.append(lst[base:base + take])
        base += take
    return out


def _build(seq: int, nb: int) -> bacc.Bacc:
    TS = nb * seq                 # total sequence columns (batches concatenated)
    QC = min(512, seq)            # q-chunk width for attention
    NQC = seq // QC               # q-chunks per batch
    NKTB = seq // 128             # k-tiles per batch
    NET = E // 128                # contraction tiles = 8
    PCB = min(512, seq)           # per-batch projection s-chunk
    NPCB = seq // PCB
    NSTB = seq // 128             # s-tiles per batch

    nc = bacc.Bacc(
        "TRN2",
        target_bir_lowering=False,
        debug=False,
        enable_asserts=False,
        num_devices=N_CORES,
    )

    xT_d = nc.dram_tensor("xT", [E, TS], BF16, kind="ExternalInput").ap()
    wq_d = nc.dram_tensor("wqT", [E, 128], BF16, kind="ExternalInput").ap()
    wk_d = nc.dram_tensor("wkT", [E, 128], BF16, kind="ExternalInput").ap()
    wv_d = nc.dram_tensor("wvT", [E, 128], BF16, kind="ExternalInput").ap()
    wo_d = nc.dram_tensor("woT", [128, E], BF16, kind="ExternalInput").ap()
    cos_d = nc.dram_tensor("cosT", [128, seq], BF16, kind="ExternalInput").ap()
    sin_d = nc.dram_tensor("sinT", [128, seq], BF16, kind="ExternalInput").ap()
    tri_d = nc.dram_tensor("tri", [128, 128], BF16, kind="ExternalInput").ap()
    out_d = nc.dram_tensor("out_p", [TS, E], F32, kind="ExternalOutput").ap()
    _nqc = seq // min(512, seq)
    _qc = min(512, seq)
    recd = [nc.dram_tensor(f"rec_scratch{b}", [2 * _nqc, _qc], F32).ap()
            for b in range(nb)]

    with tile.TileContext(nc) as tc:
        with (
            tc.tile_pool(name="persist", bufs=1) as persist,
            tc.tile_pool(name="pt", bufs=NKTB) as ptp,
            tc.tile_pool(name="ob", bufs=4) as obp,
            tc.tile_pool(name="rec", bufs=4) as recp,
            tc.tile_pool(name="ps_big", bufs=2, space="PSUM") as psb,
            tc.tile_pool(name="ps_ov", bufs=2, space="PSUM") as psov,
            tc.tile_pool(name="ps_e", bufs=2, space="PSUM") as pse,
        ):
            def T(shape, dtype, name):
                return persist.tile(shape, dtype, name=name, tag=name)

            # ---- constants / weights
            wq_s = T([128, NET, 128], BF16, "wq_s")
            wk_s = T([128, NET, 128], BF16, "wk_s")
            wv_s = T([128, NET, 128], BF16, "wv_s")
            wo_s = T([128, E], BF16, "wo_s")
            cos_s = T([128, seq], BF16, "cos_s")
            sin_s = T([128, seq], BF16, "sin_s")
            tri_s = T([128, 128], BF16, "tri_s")
            nc.sync.dma_start(out=wq_s, in_=wq_d.rearrange("(t p) d -> p t d", p=128))
            nc.sync.dma_start(out=wk_s, in_=wk_d.rearrange("(t p) d -> p t d", p=128))
            nc.sync.dma_start(out=wv_s, in_=wv_d.rearrange("(t p) d -> p t d", p=128))
            nc.sync.dma_start(out=wo_s, in_=wo_d)
            nc.sync.dma_start(out=cos_s, in_=cos_d)
            nc.sync.dma_start(out=sin_s, in_=sin_d)
            nc.sync.dma_start(out=tri_s, in_=tri_d)

            # ---- PE warm-up while input DMAs stream (HAM ramps at ~3.4us
            # of sustained activity; these dummy matmuls buy 2.4GHz for the
            # projection phase)
            wu = psb.tile([128, 512], F32, tag="psb", name="warmup")

            def warm(n):
                for _ in range(n):
                    nc.tensor.matmul(
                        wu, lhsT=wq_s[:, 0, :],
                        rhs=wq_s[:, :, :].rearrange("p t d -> p (t d)")[:, 0:512],
                        start=True, stop=True)

            warm(24)

            # ---- resident input, split into per-chunk column tiles so the
            # first projection is gated on ~1MB, not the whole input
            xts4 = {}
            for b in range(nb):
                for pc in range(NPCB):
                    for et in range(NET):
                        xt = T([128, PCB], BF16, f"xt{et}_{b}_{pc}")
                        nc.sync.dma_start(
                            out=xt,
                            in_=xT_d[et * 128:(et + 1) * 128,
                                     b * seq + pc * PCB:
                                     b * seq + (pc + 1) * PCB])
                        xts4[(et, b, pc)] = xt

            qT = T([128, TS], BF16, "qT")
            kT = T([128, TS], BF16, "kT")
            vo = T([128, nb * NKTB, 130], BF16, "vo")   # [vA|1|vB|1] per k-tile
            oT = T([128, TS], BF16, "oT")
            nc.gpsimd.memset(vo, 1.0)

            # ---------- emission helpers ----------
            def proj_qk_pieces(b, pc):
                """Micro-tasks (~2 MMs each) for one q/k projection chunk."""
                cols = slice(b * seq + pc * PCB, b * seq + (pc + 1) * PCB)
                xcols = slice(pc * PCB, (pc + 1) * PCB)
                pieces = []
                state = {}
                for wi, (w_s, dst) in enumerate(((wq_s, qT), (wk_s, kT))):
                    for e0 in range(0, NET, 2):
                        def piece(wi=wi, w_s=w_s, dst=dst, e0=e0):
                            if e0 == 0:
                                state[wi] = pse.tile(
                                    [128, PCB], F32, tag="pse",
                                    name=f"qk{b}_{pc}_{wi}")
                            ps = state[wi]
                            for et in (e0, e0 + 1):
                                nc.tensor.matmul(
                                    ps, lhsT=w_s[:, et, :],
                                    rhs=xts4[(et, b, pc)],
                                    start=(et == 0), stop=(et == NET - 1),
                                )
                            if e0 + 2 == NET:
                                if b == 0:
                                    nc.scalar.copy(out=dst[:, cols], in_=ps)
                                else:
                                    nc.vector.tensor_copy(out=dst[:, cols],
                                                          in_=ps)
                                if wi == 1:
                                    rope(b, pc)
                        pieces.append(piece)
                return pieces

            def proj_qk(b, pc):
                for p in proj_qk_pieces(b, pc):
                    p()

            def proj_v_pieces(b, st):
                gst = b * NSTB + st
                state = {}
                pieces = []
                for e0 in range(0, NET, 4):
                    def piece(e0=e0):
                        if e0 == 0:
                            state[0] = pse.tile([128, 128], F32, tag="pse",
                                                name=f"v{gst}")
                        ps = state[0]
                        per = PCB // 128
                        vpc = st // per
                        vc0 = (st % per) * 128
                        for et in range(e0, e0 + 4):
                            nc.tensor.matmul(
                                ps,
                                lhsT=xts4[(et, b, vpc)][:, vc0:vc0 + 128],
                                rhs=wv_s[:, et, :],
                                start=(et == 0), stop=(et == NET - 1),
                            )
                        if e0 + 4 == NET:
                            base = vo[:, gst, :]
                            dst = bass.AP(
                                tensor=base.tensor, offset=base.offset,
                                ap=[list(base.ap[0]), [65, 2], [1, 64]])
                            nc.vector.tensor_copy(
                                out=dst,
                                in_=ps.rearrange("p (h d) -> p h d", d=64))
                    pieces.append(piece)
                return pieces

            def proj_v(b, st):
                for p in proj_v_pieces(b, st):
                    p()

            def rope(b, pc=None):
                pcs = range(NPCB) if pc is None else [pc]
                for t, nm in ((qT, "q"), (kT, "k")):
                    for p in pcs:
                        cols = slice(b * seq + p * PCB, b * seq + (p + 1) * PCB)
                        tcols = slice(p * PCB, (p + 1) * PCB)
                        sh = recp.tile([128, PCB], BF16, tag="ropesh",
                                       name=f"sh{nm}{b}_{p}")
                        nc.vector.stream_shuffle(
                            sh, t[:, cols], [i ^ 1 for i in range(32)])
                        nc.vector.tensor_mul(sh, sh, sin_s[:, tcols])
                        nc.vector.tensor_mul(t[:, cols], t[:, cols], cos_s[:, tcols])
                        nc.vector.tensor_add(t[:, cols], t[:, cols], sh)

            pts_cache = {}

            def d1_kj(b, c, kj):
                qbase = c * QC
                gq0 = b * seq + qbase
                o = kj * 128 - qbase
                ro = max(o, 0)
                nj = QC - ro
                kc = b * seq + kj * 128
                ps = psb.tile([128, 2, QC], F32, tag="psb",
                              name=f"ss{b}_{c}_{kj}")
                for h in range(2):
                    rows = slice(h * 64, h * 64 + 64)
                    nc.tensor.matmul(
                        ps[:, h, 0:nj],
                        lhsT=kT[rows, kc:kc + 128],
                        rhs=qT[rows, gq0 + ro:gq0 + QC],
                        start=True, stop=True,
                        tile_position=(h * 64, 0),
                    )
                pt = ptp.tile([128, 2, QC], BF16, tag="pt",
                              name=f"pt{b}_{c}_{kj}")
                nc.scalar.activation(
                    pt[:, :, 0:nj], ps[:, :, 0:nj],
                    mybir.ActivationFunctionType.Exp,
                )
                if o >= 0:
                    tri_b = bass.AP(
                        tensor=tri_s.tensor, offset=tri_s.offset,
                        ap=[list(tri_s.ap[0]), [0, 2], list(tri_s.ap[1])],
                    )
                    nc.gpsimd.tensor_mul(
                        pt[:, :, 0:128], pt[:, :, 0:128], tri_b)
                return pt, ro, nj

            def attn_chunk(b, c, fills):
                qbase = c * QC
                gq0 = b * seq + qbase
                nkt = (qbase + QC) // 128
                ops_ = [psov.tile([65, QC], F32, tag="psov", name=f"o{b}_{c}_{h}")
                        for h in range(2)]
                fq = []
                for kind, idx in fills:
                    if kind == "qk0":
                        fq.extend(proj_qk_pieces(0, idx))
                    elif kind == "v0":
                        fq.extend(proj_v_pieces(0, idx))
                    elif kind == "qk":
                        fq.extend(proj_qk_pieces(1, idx))
                    elif kind == "v":
                        fq.extend(proj_v_pieces(1, idx))
                    else:
                        fq.extend(eproj_pieces(idx))
                for kj in range(nkt):
                    pt, ro, nj = d1_kj(b, c, kj)
                    if kj >= 2:
                        d2_kj(b, c, kj - 2, ops_, nkt)
                    for _ in range(2):
                        if fq:
                            fq.pop(0)()
                    pts_cache[(b, c, kj)] = (pt, ro, nj)
                for kj in range(max(nkt - 2, 0), nkt):
                    d2_kj(b, c, kj, ops_, nkt)
                for p in fq:
                    p()
                return ops_

            def d2_kj(b, c, kj, ops_, nkt):
                pt, ro, nj = pts_cache[(b, c, kj)]
                for h in range(2):
                    nc.tensor.matmul(
                        ops_[h][:, ro:QC],
                        lhsT=vo[:, b * NKTB + kj, h * 65:h * 65 + 65],
                        rhs=pt[:, h, 0:nj],
                        start=(kj == 0), stop=(kj == nkt - 1),
                    )

            def d3(b, c, ops_):
                gq0 = b * seq + c * QC
                dch = recp.tile([64, QC], F32, tag="dch", name=f"dch{b}_{c}")
                nc.gpsimd.memset(dch, 1.0)
                for h in range(2):
                    op = ops_[h]
                    nc.vector.tensor_copy(
                        out=oT[h * 64:h * 64 + 64, gq0:gq0 + QC],
                        in_=op[0:64, 0:QC])
                    nc.vector.tensor_copy(
                        out=dch[32 * h:32 * h + 1, :],
                        in_=op[64:65, 0:QC])
                return dch

            def norm_chunk(b, c, dch):
                gq0 = b * seq + c * QC
                rec = recp.tile([64, QC], F32, tag="rec", name=f"rca{b}_{c}")
                lg = recp.tile([64, QC], F32, tag="lg", name=f"lg{b}_{c}")
                nc.scalar.activation(lg, dch,
                                     mybir.ActivationFunctionType.Ln)
                nc.scalar.activation(rec, lg,
                                     mybir.ActivationFunctionType.Exp,
                                     scale=-1.0)
                for h in range(2):
                    nc.sync.dma_start(
                        out=recd[b][2 * c + h:2 * c + h + 1, :],
                        in_=rec[32 * h:32 * h + 1, :])
                rb = recp.tile([128, QC], F32, tag="rb", name=f"rb{b}_{c}")
                for h in range(2):
                    row = recd[b][2 * c + h:2 * c + h + 1, :]
                    bcast = bass.AP(tensor=row.tensor, offset=row.offset,
                                    ap=[[0, 64], [1, QC]])
                    nc.sync.dma_start(out=rb[h * 64:h * 64 + 64, :], in_=bcast)
                nc.vector.tensor_mul(
                    oT[:, gq0:gq0 + QC], oT[:, gq0:gq0 + QC], rb)

            def eproj_pieces(gst):
                return [lambda ec=ec: eproj_one(gst, ec) for ec in range(E // 512)]

            def eproj(gst):
                for ec in range(E // 512):
                    eproj_one(gst, ec)

            def eproj_one(gst, ec):
                if True:
                    ps = pse.tile([128, 512], F32, tag="pse", name=f"op{gst}_{ec}")
                    nc.tensor.matmul(
                        ps,
                        lhsT=oT[:, gst * 128:(gst + 1) * 128],
                        rhs=wo_s[:, ec * 512:(ec + 1) * 512],
                        start=True, stop=True,
                    )
                    ob = obp.tile([128, 512], F32, tag="ob", name=f"ob{gst}_{ec}")
                    if gst < NSTB:
                        nc.vector.tensor_copy(out=ob, in_=ps)
                    else:
                        nc.scalar.copy(out=ob, in_=ps)
                    nc.sync.dma_start(
                        out=out_d[gst * 128:(gst + 1) * 128, ec * 512:(ec + 1) * 512],
                        in_=ob,
                    )

            # ---------- emission ----------
            for pc in range(NPCB):
                proj_qk(0, pc)
            for st in range(NSTB):
                proj_v(0, st)

            fillers = {c: [] for c in range(NQC)}
            if nb > 1:
                for i, pc in enumerate(range(NPCB)):
                    fillers[min(i, NQC - 1)].append(("qk", pc))
                for i, grp in enumerate(_split(list(range(NSTB)),
                                               max(NQC - 1, 1))):
                    fillers[min(i + 1, NQC - 1)].extend(("v", st) for st in grp)

            for c in range(NQC):
                ops_ = attn_chunk(0, c, fillers.get(c, []))
                dch = d3(0, c, ops_)
                norm_chunk(0, c, dch)

            if nb > 1:
                # batch-1 attention; batch-0 out-proj as PE filler,
                # batch-1 out-proj one chunk behind
                e0_fill = _split(list(range(NSTB)), NQC)
                spc = QC // 128
                cs = list(reversed(range(NQC)))
                for i, c in enumerate(cs):
                    fills = [("e", st) for st in e0_fill[i]]
                    if i >= 1:
                        pc_ = cs[i - 1]
                        fills += [("e", NSTB + st)
                                  for st in range(pc_ * spc, (pc_ + 1) * spc)]
                    ops_ = attn_chunk(1, c, fills)
                    dch = d3(1, c, ops_)
                    norm_chunk(1, c, dch)
                lc = cs[-1]
                for st in range(lc * spc, (lc + 1) * spc):
                    eproj(NSTB + st)
            else:
                for st in range(NSTB):
                    eproj(st)

    nc.compile()
    return nc


@functools.lru_cache(maxsize=2)
def _built(seq: int, nb: int) -> bacc.Bacc:
    return _build(seq, nb)


def _host_tables(seq: int):
    inv = 1.0 / (ROPE_BASE ** (np.arange(0, HD, 2, dtype=np.float32) / HD))
    f = np.outer(np.arange(seq, dtype=np.float32), inv)
    emb = np.concatenate([f, f], axis=-1)        # [S, 64] (concat layout)
    cos = np.cos(emb).T.astype(np.float32)       # [64, S]
    sin = np.sin(emb).T.astype(np.float32)
    sgn = np.where(np.arange(HD) % 2 == 0, -1.0, 1.0).astype(np.float32)
    sin_signed = sin * sgn[:, None]
    cosT = np.concatenate([cos, cos], axis=0).astype(BF)       # [128, S]
    sinT = np.concatenate([sin_signed, sin_signed], axis=0).astype(BF)
    return cosT, sinT


def make_in_maps(x, Wq, Wk, Wv, Wo):
    x = np.asarray(x, dtype=np.float32)
    B, S, E_ = x.shape
    assert E_ == E
    xT = np.ascontiguousarray(x.reshape(B * S, E_).T).astype(BF)   # [E, B*S]
    cosT, sinT = _host_tables(S)
    i_idx = np.arange(128)
    tri = (i_idx[None, :] >= i_idx[:, None]).astype(BF)  # keep j >= i
    scale = np.float32(HD ** -0.5)
    in_maps = []
    for core in range(N_CORES):
        cols = slice(core * 128, core * 128 + 128)   # heads 2c, 2c+1 dims
        wqT = np.ascontiguousarray((np.asarray(Wq)[cols, :] * scale).T).astype(BF)
        wkT = np.ascontiguousarray(np.asarray(Wk)[cols, :].T).astype(BF)
        wvT = np.ascontiguousarray(np.asarray(Wv)[cols, :].T).astype(BF)
        woT = np.ascontiguousarray(np.asarray(Wo)[:, cols].T).astype(BF)
        in_maps.append(dict(
            xT=xT, wqT=wqT, wkT=wkT, wvT=wvT, woT=woT,
            cosT=cosT, sinT=sinT, tri=tri,
        ))
    return in_maps


def kernel(x, Wq, Wk, Wv, Wo):
    x = np.asarray(x, dtype=np.float32)
    B, S, E_ = x.shape
    nc = _built(S, B)
    in_maps = make_in_maps(x, Wq, Wk, Wv, Wo)
    res = run_bass_kernel_spmd(nc, in_maps, core_ids=list(range(N_CORES)))
    out = np.zeros((B * S, E_), np.float32)
    for r in res.results:
        out += r["out_p"]
    return out.reshape(B, S, E_)
